# revision 9
# baseline (speedup 1.0000x reference)
"""Trainium2 Bass kernel for nn_Attention3D (B=4, C=256, D=H=W=16).

y = x + wp @ softmax_j((wq@x+bq)^T (wk@x+bk) / sqrt(C)) applied to (wv@x+bv), + bp

Sharding: 8 cores = (batch b, query-half). Each core owns one batch's full
K/V (N=4096 keys) and half the queries (NI=2048). Key order is permuted per
core so "my" queries are always columns 0:2048 — softmax/attention are
invariant to key permutation, so every core runs the identical program.

v2 design notes (vs v1):
  - All projections in fp8 e4m3 DoubleRow (virtual K=256 in one matmul):
    4x fewer projection PE cycles than bf16, and x ships as fp8 (1MB DMA).
  - bk is DROPPED: scores pick up q_i . bk, constant over keys j, which
    cancels exactly in softmax.  bv is folded into the output bias
    (softmax rows sum to 1): bias_out = bp + wp@bv.
  - v-projection and out-projection fused: wvp = (wp@wv)^T, built directly
    in [key, channel] stationary layout so the attention matmul emits the
    projected output with no transposes.
  - Attention per i-chunk of 1024 queries, software-pipelined (scores/exp
    run 2-3 superblocks ahead of the v-matmuls) with the remaining k/v/q
    projections interleaved as PE fill work.
  - Softmax denominator: ones-matmul batches of 4 superblocks (DoubleRow,
    one transient PSUM tile, single LDWEIGHTS) accumulated into SBUF by
    VectorE; reciprocal + normalize on VectorE; bias+residual on GpSimd.
  - PSUM: scores ring 2x[128,1024]f32 (4 banks) + 2 attention accumulators
    (4 banks) = 8 banks exactly; S batches time-share the scores ring.
"""

import numpy as np
import ml_dtypes

B, C = 4, 256
P = 128
D = H = W = 16
N = D * H * W          # 4096 voxels
NI = 2048              # queries per core
NCORES = 8
IC = 1024              # i-chunk
HIC = 512              # half chunk (one PSUM bank of fp32)
NICH = NI // IC        # 2 i-chunks
NJ = N // P            # 32 key blocks
NSB = NJ // 2          # 16 key superblocks (256 keys each, fp8 DoubleRow)
ESHIFT = -4 * 0.6931471805599453  # exp bias: fold 2^-4 so e fits fp8 e4m3
SCALE = float(C) ** -0.5

_cache = {}


def _build():
    import concourse.bacc as bacc
    import concourse.mybir as mybir
    import concourse.tile as tile

    dt = mybir.dt
    f32, f8 = dt.float32, dt.float8e4

    nc = bacc.Bacc("TRN2", target_bir_lowering=False, debug=False)

    # [ci, pair*N + n] channel c = pair*128 + ci (pair-interleaved for DR)
    x8_d = nc.dram_tensor("x8", [P, 2 * N], f8, kind="ExternalInput")
    w_d = {
        w: nc.dram_tensor(w, [P, 2 * C], f8, kind="ExternalInput")
        for w in ("wq8", "wk8", "wvp8")
    }
    bq_d = nc.dram_tensor("bq", [C, 1], f32, kind="ExternalInput")
    xq_d = nc.dram_tensor("xq32", [C, NI], f32, kind="ExternalInput")
    y_d = nc.dram_tensor("y", [C, NI], f32, kind="ExternalOutput")

    add = mybir.AluOpType.add
    EXP = mybir.ActivationFunctionType.Exp
    DR = mybir.MatmulPerfMode.DoubleRow

    with tile.TileContext(nc) as tc:
        with (
            tc.tile_pool(name="consts", bufs=1) as consts,
            tc.tile_pool(name="acts", bufs=1) as acts,
            tc.tile_pool(name="e16p", bufs=8) as e16p,
            tc.tile_pool(name="sm", bufs=2) as sm,
            tc.tile_pool(name="ys", bufs=3) as ys,
            tc.tile_pool(name="ps_s", bufs=2, space="PSUM") as ps_s,
            tc.tile_pool(name="ps_a", bufs=1, space="PSUM") as ps_a,
        ):
            # ---- constants ----
            bq_sb = []
            for ob in range(2):
                t = consts.tile([P, 1], f32, tag=f"bq{ob}", name=f"bq{ob}")
                nc.gpsimd.dma_start(out=t, in_=bq_d.ap()[ob * P:(ob + 1) * P, :])
                bq_sb.append(t)
            eshift_t = consts.tile([P, 1], f32, tag="eshift")
            nc.vector.memset(eshift_t, ESHIFT)
            ones8 = consts.tile([P, 2, P], f8, tag="ones8")
            nc.vector.memset(ones8, 1.0)

            # ---- weights + input DMAs (wq/wk first; x8 split across queues) ----
            w_sb = {}
            for wname in ("wq8", "wk8"):
                t = consts.tile([P, 2, C], f8, tag=wname, name=wname)
                for pair in range(2):
                    eng = nc.sync if pair == 0 else nc.gpsimd
                    eng.dma_start(out=t[:, pair, :],
                                  in_=w_d[wname].ap()[:, pair * C:(pair + 1) * C])
                w_sb[wname] = t
            x8 = acts.tile([P, 2, N], f8, tag="x8")
            for ch in range(4):
                c0, c1 = ch * (N // 4), (ch + 1) * (N // 4)
                for pair in range(2):
                    eng = nc.sync if pair == 0 else nc.gpsimd
                    eng.dma_start(out=x8[:, pair, c0:c1],
                                  in_=x8_d.ap()[:, pair * N + c0:pair * N + c1])
            wvp8 = consts.tile([P, 2, C], f8, tag="wvp8")
            for pair in range(2):
                eng = nc.sync if pair == 0 else nc.gpsimd
                eng.dma_start(out=wvp8[:, pair, :],
                              in_=w_d["wvp8"].ap()[:, pair * C:(pair + 1) * C])
            xq32 = acts.tile([P, 2, NI], f32, tag="xq32")
            for ob in range(2):
                nc.gpsimd.dma_start(out=xq32[:, ob, :],
                                    in_=xq_d.ap()[ob * P:(ob + 1) * P, :])

            # ---- activations ----
            q16 = acts.tile([P, 2, NI], f8, tag="q16")      # [ci, pair, i]
            k16 = acts.tile([P, 2, N], f8, tag="k16")       # [ci, pair, j]
            vT8 = acts.tile([P, NJ, C], f8, tag="vT8")      # [ji, jb, c]

            # ---- projection emitters (fp8 DR: full K=256 in one matmul) ----
            def emit_qproj(icnk):
                for ob in range(2):
                    ps = ps_s.tile([P, IC], f32, tag="ps", name="qps")
                    for h in range(2):
                        i0 = icnk * IC + h * HIC
                        nc.tensor.matmul(
                            ps[:, h * HIC:(h + 1) * HIC],
                            w_sb["wq8"][:, :, ob * P:(ob + 1) * P],
                            x8[:, :, i0:i0 + HIC],
                            start=True, stop=True, perf_mode=DR)
                    nc.vector.tensor_scalar_add(
                        q16[:, ob, icnk * IC:(icnk + 1) * IC], ps, bq_sb[ob])

            def emit_kproj(jc):
                for ob in range(2):
                    ps = ps_s.tile([P, IC], f32, tag="ps", name="kps")
                    for h in range(2):
                        j0 = jc * IC + h * HIC
                        nc.tensor.matmul(
                            ps[:, h * HIC:(h + 1) * HIC],
                            w_sb["wk8"][:, :, ob * P:(ob + 1) * P],
                            x8[:, :, j0:j0 + HIC],
                            start=True, stop=True, perf_mode=DR)
                    nc.vector.tensor_copy(k16[:, ob, jc * IC:(jc + 1) * IC], ps)

            def emit_vproj(g):   # group of 4 key blocks
                ps = ps_s.tile([P, IC], f32, tag="ps", name="vps")
                for jj in range(4):
                    jb = 4 * g + jj
                    nc.tensor.matmul(
                        ps[:, jj * C:(jj + 1) * C],
                        x8[:, :, jb * P:(jb + 1) * P], wvp8,
                        start=True, stop=True, perf_mode=DR)
                nc.vector.tensor_copy(vT8[:, 4 * g:4 * g + 4, :], ps)

            # ---- attention emitters ----
            es = {}

            def scores_exp(icnk, sb):
                e16 = e16p.tile([P, 2, IC], f8, tag="e16", name="e16")
                for r in range(2):
                    jb = 2 * sb + r
                    sps = ps_s.tile([P, IC], f32, tag="ps", name="sps")
                    for h in range(2):
                        i0 = icnk * IC + h * HIC
                        nc.tensor.matmul(
                            sps[:, h * HIC:(h + 1) * HIC],
                            k16[:, :, jb * P:(jb + 1) * P],
                            q16[:, :, i0:i0 + HIC],
                            start=True, stop=True, perf_mode=DR)
                    nc.scalar.activation(e16[:, r, :], sps, EXP,
                                         scale=SCALE, bias=eshift_t)
                es[sb] = e16

            def attn(sb, a_ps):
                e16 = es[sb]
                first, last = (sb == 0), (sb == NSB - 1)
                for cb in range(2):
                    for h in range(2):
                        nc.tensor.matmul(
                            a_ps[cb][:, h * HIC:(h + 1) * HIC],
                            vT8[:, 2 * sb:2 * sb + 2, cb * P:(cb + 1) * P],
                            e16[:, :, h * HIC:(h + 1) * HIC],
                            start=first, stop=last, perf_mode=DR)

            def sbatch(sb_hi, Sacc):
                sb_ps = ps_s.tile([P, IC], f32, tag="ps", name="sbps")
                group = [es.pop(s) for s in range(sb_hi - 3, sb_hi + 1)]
                for bi, et in enumerate(group):
                    for h in range(2):
                        nc.tensor.matmul(
                            sb_ps[:, h * HIC:(h + 1) * HIC], ones8,
                            et[:, :, h * HIC:(h + 1) * HIC],
                            start=(bi == 0), stop=(bi == 3), perf_mode=DR)
                if sb_hi == 3:
                    nc.vector.tensor_copy(Sacc, sb_ps)
                else:
                    nc.vector.tensor_add(Sacc, Sacc, sb_ps)

            # ---- schedule ----
            # minimal prologue: q(ic0) + k(jc0) unlock the first scores
            emit_qproj(0)
            emit_kproj(0)
            # PE fill work popped one item per superblock during ic0
            fills = [lambda: emit_vproj(0), lambda: emit_vproj(1),
                     lambda: emit_kproj(1), lambda: emit_vproj(2),
                     lambda: emit_kproj(2), lambda: emit_vproj(3),
                     lambda: emit_kproj(3), lambda: emit_vproj(4),
                     lambda: emit_qproj(1), lambda: emit_vproj(5),
                     lambda: emit_vproj(6), lambda: emit_vproj(7)]

            for icnk in range(NICH):
                isl = slice(icnk * IC, (icnk + 1) * IC)
                a_ps = [ps_a.tile([P, IC], f32, tag=f"a{cb}", name=f"aps{cb}")
                        for cb in range(2)]
                Sacc = sm.tile([P, IC], f32, tag="Sacc")
                DEPTH = 3
                for sb in range(NSB):
                    scores_exp(icnk, sb)
                    if icnk == 0 and fills:
                        fills.pop(0)()
                    if sb >= DEPTH:
                        attn(sb - DEPTH, a_ps)
                        if (sb - DEPTH) % 4 == 3:
                            sbatch(sb - DEPTH, Sacc)
                for sb in range(NSB - DEPTH, NSB):
                    attn(sb, a_ps)
                    if sb % 4 == 3:
                        sbatch(sb, Sacc)

                # epilogue: R = 1/S; y = a*R + bias_out + x
                R = sm.tile([P, IC], f32, tag="R")
                nc.vector.reciprocal_approx_fast(out=R, in_=Sacc)
                for ob in range(2):
                    tmp = ys.tile([P, IC], f32, tag="tmp")
                    nc.vector.tensor_mul(tmp, a_ps[ob], R)
                    yt = ys.tile([P, IC], f32, tag="yt")
                    # xq32 holds x + bias_out (pre-added on host)
                    nc.gpsimd.tensor_add(yt, tmp, xq32[:, ob, isl])
                    nc.sync.dma_start(out=y_d.ap()[ob * P:(ob + 1) * P, isl],
                                      in_=yt)

    nc.compile()
    return nc


def _pack_pairs(a):
    """[C, M] row-major -> [P, 2*M] with row ci holding (pair0 cols, pair1 cols),
    channel c = pair*128 + ci."""
    Cc, M = a.shape
    return np.ascontiguousarray(
        a.reshape(2, P, M).transpose(1, 0, 2).reshape(P, 2 * M))


def _prep_inputs(x, wq, bq, wk, bk, wv, bv, wp, bp):
    f8 = ml_dtypes.float8_e4m3fn
    xf = np.asarray(x, np.float32).reshape(B, C, N)
    wq64 = np.asarray(wq, np.float64)
    wk64 = np.asarray(wk, np.float64)
    wv64 = np.asarray(wv, np.float64)
    wp64 = np.asarray(wp, np.float64)
    # out-projection folded into the v-projection: wp @ (v·p) == (wvp^T x)·p
    wvp = (wp64 @ wv64).astype(np.float32)
    # softmax rows sum to 1 -> bv contributes wp@bv to every output column
    bout = (np.asarray(bp, np.float64)
            + wp64 @ np.asarray(bv, np.float64)).astype(np.float32)
    shared = {
        "wq8": _pack_pairs(np.asarray(wq, np.float32).T).astype(f8),
        "wk8": _pack_pairs(np.asarray(wk, np.float32).T).astype(f8),
        "wvp8": _pack_pairs(wvp.T).astype(f8),
        "bq": np.asarray(bq, np.float32).reshape(C, 1),
    }
    in_maps = []
    for core in range(NCORES):
        b, h = core // 2, core % 2
        xs = xf[b]
        if h == 1:  # roll so this core's query half is first (key order irrelevant)
            xs = np.concatenate([xs[:, NI:], xs[:, :NI]], axis=1)
        m = dict(shared)
        m["x8"] = _pack_pairs(xs).astype(f8)
        # residual with the output bias folded in: y = a*R + (x + bout)
        m["xq32"] = np.ascontiguousarray(
            xs[:, :NI] + bout[:, None], np.float32)
        in_maps.append(m)
    return in_maps


def _run(inputs, trace=False, **kwargs):
    from concourse.bass_utils import run_bass_kernel_spmd

    if "nc" not in _cache:
        _cache["nc"] = _build()
    nc = _cache["nc"]
    in_maps = _prep_inputs(**inputs)
    res = run_bass_kernel_spmd(
        nc, in_maps, core_ids=list(range(NCORES)), trace=trace, **kwargs
    )
    out = np.empty((B, C, N), np.float32)
    for core in range(NCORES):
        b, h = core // 2, core % 2
        out[b][:, h * NI:(h + 1) * NI] = res.results[core]["y"]
    return out.reshape(B, C, D, H, W), res


def kernel(**inputs):
    out, _ = _run(inputs)
    return out


# revision 17
# speedup vs baseline: 1.0469x; 1.0469x over previous
"""Trainium2 Bass kernel for nn_Attention3D (B=4, C=256, D=H=W=16).

y = x + wp @ softmax_j((wq@x+bq)^T (wk@x+bk) / sqrt(C)) applied to (wv@x+bv), + bp

Sharding: 8 cores = (batch b, query-half). Each core owns one batch's full
K/V (N=4096 keys) and half the queries (NI=2048). Key order is permuted per
core so "my" queries are always columns 0:2048 — softmax/attention are
invariant to key permutation, so every core runs the identical program.

v3 design (PE matmul-count-minimal; every 512-wide MM costs ~215ns on HW):
  - q-projection ELIMINATED: scores s[i,j] = x_i^T (wq^T wk) x_j + (wk^T
    bq)^T x_j.  k'' = M x with M = wq^T wk (one fp8 DR matmul per tile);
    the bq term is a per-KEY scalar beta_j fed through the exp's
    per-partition bias operand (keys are the partition dim of scores).
  - bk dropped (constant over keys -> cancels in softmax); bv folded into
    the residual input (softmax rows sum to 1): xq16 = x + bp + wp@bv.
  - v+out projection fused (wvp = wp@wv) and split by SVD into pass A
    (components 0..127) and pass B (128..254, sigma_255 dropped ~1e-4):
    stationaries vA'(128 cols) and vB'(127 cols + a constant-1 column), so
    the softmax denominator S rides the attention accumulation on PSUM
    partition 127 — the 64 ones-matmuls of v2 vanish.  beta is produced by
    a 256th column (wk^T bq) of the same fused projection.
  - Normalized pA/pB (bf16) are then projected back by U-side stationaries
    (8 small bf16 MMs per chunk); a 2-MM fp16 broadcast matmul replicates
    S across partitions for the reciprocal.
  - Attention per i-chunk of 1024 queries, software-pipelined 3 superblocks
    deep with the k/v projections interleaved as PE fill work in chunk 0.
  - PSUM: scores ring 2x[128,1024]f32 (4 banks) + accumulators aA,aB
    (4 banks) = 8 exactly; S-broadcast/y-proj tiles time-share the ring.
"""

import numpy as np
import ml_dtypes

B, C = 4, 256
P = 128
D = H = W = 16
N = D * H * W          # 4096 voxels
NI = 2048              # queries per core
NCORES = 8
IC = 1024              # i-chunk
HIC = 512              # half chunk (one PSUM bank of fp32)
NICH = NI // IC        # 2 i-chunks
NJ = N // P            # 32 key blocks
NSB = NJ // 2          # 16 key superblocks (256 keys each, fp8 DoubleRow)
ESHIFT = -4 * 0.6931471805599453  # exp bias: fold 2^-4 so e fits fp8 e4m3
SCALE = float(C) ** -0.5

_cache = {}


def _build():
    import concourse.bacc as bacc
    import concourse.mybir as mybir
    import concourse.tile as tile

    dt = mybir.dt
    f32, f16, bf16, f8 = dt.float32, dt.float16, dt.bfloat16, dt.float8e4

    nc = bacc.Bacc("TRN2", target_bir_lowering=False, debug=False)

    # [ci, pair*N + n], channel c = pair*128 + ci (pair-interleaved for DR)
    x8_d = nc.dram_tensor("x8", [P, 2 * N], f8, kind="ExternalInput")
    mk8_d = nc.dram_tensor("mk8", [P, 2 * C], f8, kind="ExternalInput")
    wv8_d = nc.dram_tensor("wvab8", [P, 2 * C], f8, kind="ExternalInput")
    ua_d = nc.dram_tensor("uat16", [P, C], bf16, kind="ExternalInput")
    ub_d = nc.dram_tensor("ubt16", [P, C], bf16, kind="ExternalInput")
    xq_d = nc.dram_tensor("xq16", [C, NI], bf16, kind="ExternalInput")
    y_d = nc.dram_tensor("y", [C, NI], f32, kind="ExternalOutput")

    EXP = mybir.ActivationFunctionType.Exp
    DR = mybir.MatmulPerfMode.DoubleRow
    mult = mybir.AluOpType.mult
    add = mybir.AluOpType.add

    with tile.TileContext(nc) as tc:
        with (
            tc.tile_pool(name="consts", bufs=1) as consts,
            tc.tile_pool(name="acts", bufs=1) as acts,
            tc.tile_pool(name="e16p", bufs=6) as e16p,
            tc.tile_pool(name="sm", bufs=2) as sm,
            tc.tile_pool(name="ys", bufs=4) as ys,
            tc.tile_pool(name="ps_s", bufs=2, space="PSUM") as ps_s,
            tc.tile_pool(name="ps_a", bufs=1, space="PSUM") as ps_a,
        ):
            # ---- weights + input DMAs (mk/wvab first; x8 split across queues) ----
            w_sb = {}
            for wname, wd in (("mk8", mk8_d), ("wvab8", wv8_d)):
                t = consts.tile([P, 2, C], f8, tag=wname, name=wname)
                for pair in range(2):
                    eng = nc.sync if pair == 0 else nc.gpsimd
                    eng.dma_start(out=t[:, pair, :],
                                  in_=wd.ap()[:, pair * C:(pair + 1) * C])
                w_sb[wname] = t
            x8 = acts.tile([P, 2, N], f8, tag="x8")
            for ch in range(4):
                c0, c1 = ch * (N // 4), (ch + 1) * (N // 4)
                for pair in range(2):
                    eng = nc.sync if pair == 0 else nc.gpsimd
                    eng.dma_start(out=x8[:, pair, c0:c1],
                                  in_=x8_d.ap()[:, pair * N + c0:pair * N + c1])
            uat16 = consts.tile([P, C], bf16, tag="uat16")
            nc.sync.dma_start(out=uat16, in_=ua_d.ap())
            ubt16 = consts.tile([P, C], bf16, tag="ubt16")
            nc.gpsimd.dma_start(out=ubt16, in_=ub_d.ap())
            xq16 = acts.tile([P, 2, NI], bf16, tag="xq16")
            ones16f = consts.tile([1, P], f16, tag="ones16f")
            nc.vector.memset(ones16f, 1.0)

            # ---- activations ----
            k16 = acts.tile([P, 2, N], f8, tag="k16")        # [ci, pair, j]
            # [ji, jb, col]: cols 0:128 = vA, col 128 = ones (S rides the
            # B-pass on OUT PARTITION 0), cols 129:256 = vB channels
            vT8 = acts.tile([P, NJ, C], f8, tag="vT8")
            nc.vector.memset(vT8[:, :, 128], 1.0)
            beta = acts.tile([P, NJ // 4, 4], f32, tag="beta")  # exp bias / jb

            # ---- projections (fp8 DR: full K=256 contraction per matmul) ----
            def emit_kproj(jc):
                for ob in range(2):
                    ps = ps_s.tile([P, IC], f32, tag="ps", name="kps")
                    for h in range(2):
                        j0 = jc * IC + h * HIC
                        nc.tensor.matmul(
                            ps[:, h * HIC:(h + 1) * HIC],
                            w_sb["mk8"][:, :, ob * P:(ob + 1) * P],
                            x8[:, :, j0:j0 + HIC],
                            start=True, stop=True, perf_mode=DR)
                    nc.vector.tensor_copy(k16[:, ob, jc * IC:(jc + 1) * IC], ps)

            def emit_vproj(g):   # group of 4 key blocks -> vA|vB|beta columns
                psv = ps_s.tile([P, 4, C], f32, tag="ps", name="vps")
                for jj in range(4):
                    jb = 4 * g + jj
                    nc.tensor.matmul(
                        psv[:, jj, :],
                        x8[:, :, jb * P:(jb + 1) * P], w_sb["wvab8"],
                        start=True, stop=True, perf_mode=DR)
                nc.vector.tensor_copy(vT8[:, 4 * g:4 * g + 4, 0:128],
                                      psv[:, :, 0:128])
                nc.vector.tensor_copy(vT8[:, 4 * g:4 * g + 4, 129:256],
                                      psv[:, :, 128:255])
                nc.vector.tensor_scalar(beta[:, g, :], psv[:, :, 255],
                                        SCALE, ESHIFT, op0=mult, op1=add)

            # ---- attention emitters ----
            es = {}

            def scores_exp(icnk, sb):
                e16 = e16p.tile([P, 2, IC], f8, tag="e16", name="e16")
                for r in range(2):
                    jb = 2 * sb + r
                    sps = ps_s.tile([P, IC], f32, tag="ps", name="sps")
                    for h in range(2):
                        i0 = icnk * IC + h * HIC
                        nc.tensor.matmul(
                            sps[:, h * HIC:(h + 1) * HIC],
                            k16[:, :, jb * P:(jb + 1) * P],
                            x8[:, :, i0:i0 + HIC],
                            start=True, stop=True, perf_mode=DR)
                    nc.scalar.activation(e16[:, r, :], sps, EXP, scale=SCALE,
                                         bias=beta[:, jb // 4, jb % 4:jb % 4 + 1])
                es[icnk, sb] = e16

            def attn(icnk, sb, a_ps):
                e16 = es.pop((icnk, sb))
                first, last = (sb == 0), (sb == NSB - 1)
                for pi in range(2):     # pass A (chans 0:128) / B (S + 128:255)
                    for h in range(2):
                        nc.tensor.matmul(
                            a_ps[pi][:, h * HIC:(h + 1) * HIC],
                            vT8[:, 2 * sb:2 * sb + 2, pi * P:(pi + 1) * P],
                            e16[:, :, h * HIC:(h + 1) * HIC],
                            start=first, stop=last, perf_mode=DR)

            # ---- schedule ----
            emit_kproj(0)
            emit_vproj(0)
            emit_vproj(1)
            fills = [lambda: emit_vproj(2), lambda: emit_kproj(1),
                     lambda: emit_vproj(3), lambda: emit_vproj(4),
                     lambda: emit_kproj(2), lambda: emit_vproj(5),
                     lambda: emit_vproj(6), lambda: emit_kproj(3),
                     lambda: emit_vproj(7)]
            DEPTH = 3

            for icnk in range(NICH):
                isl = slice(icnk * IC, (icnk + 1) * IC)
                for ob in range(2):   # residual arrives late; keep DMA off the
                    nc.gpsimd.dma_start(   # critical input window
                        out=xq16[:, ob, isl],
                        in_=xq_d.ap()[ob * P:(ob + 1) * P, isl])
                a_ps = [ps_a.tile([P, IC], f32, tag=f"a{pi}", name=f"aps{pi}")
                        for pi in range(2)]
                for sb in range(NSB):
                    if (icnk, sb) not in es:   # ic1's first DEPTH pre-rolled
                        scores_exp(icnk, sb)
                    if icnk == 0 and fills:
                        fills.pop(0)()
                    if sb >= DEPTH:
                        attn(icnk, sb - DEPTH, a_ps)
                for sb in range(NSB - DEPTH, NSB):
                    attn(icnk, sb, a_ps)
                # pre-roll the next chunk's first scores so PE/Act stay busy
                # through this chunk's epilogue
                if icnk + 1 < NICH:
                    for sb in range(DEPTH):
                        scores_exp(icnk + 1, sb)

                # ---- epilogue: S-broadcast, R, normalize, project back ----
                S16 = sm.tile([1, IC], f16, tag="S16")
                nc.vector.tensor_copy(S16, a_ps[1][0:1, :])
                sb_ps = ps_s.tile([P, IC], f32, tag="ps", name="sbps")
                for h in range(2):
                    nc.tensor.matmul(sb_ps[:, h * HIC:(h + 1) * HIC],
                                     ones16f, S16[:, h * HIC:(h + 1) * HIC],
                                     start=True, stop=True)
                R = sm.tile([P, IC], f32, tag="R")
                nc.vector.reciprocal_approx_fast(out=R, in_=sb_ps)
                p16 = [ys.tile([P, IC], bf16, tag=f"p{pi}", name=f"p{pi}")
                       for pi in range(2)]
                for pi in range(2):
                    nc.vector.tensor_mul(p16[pi], a_ps[pi], R)
                for cb in range(2):
                    yp = ps_s.tile([P, IC], f32, tag="ps", name="yps")
                    for h in range(2):
                        for pi, u in ((0, uat16), (1, ubt16)):
                            nc.tensor.matmul(
                                yp[:, h * HIC:(h + 1) * HIC],
                                u[:, cb * P:(cb + 1) * P],
                                p16[pi][:, h * HIC:(h + 1) * HIC],
                                start=(pi == 0), stop=(pi == 1))
                    yt = ys.tile([P, IC], f32, tag="yt")
                    # xq16 holds x + bp + wp@bv (pre-added on host)
                    nc.vector.tensor_add(yt, yp, xq16[:, cb, isl])
                    nc.sync.dma_start(out=y_d.ap()[cb * P:(cb + 1) * P, isl],
                                      in_=yt)

    nc.compile()
    return nc


def _pack_pairs(a):
    """[C, M] -> [P, 2*M]: row ci holds (pair0 cols, pair1 cols),
    channel c = pair*128 + ci."""
    Cc, M = a.shape
    return np.ascontiguousarray(
        a.reshape(2, P, M).transpose(1, 0, 2).reshape(P, 2 * M))


def _prep_inputs(x, wq, bq, wk, bk, wv, bv, wp, bp):
    f8 = ml_dtypes.float8_e4m3fn
    bf = ml_dtypes.bfloat16
    xf = np.asarray(x, np.float32).reshape(B, C, N)
    wq64 = np.asarray(wq, np.float64)
    wk64 = np.asarray(wk, np.float64)
    wv64 = np.asarray(wv, np.float64)
    wp64 = np.asarray(wp, np.float64)
    bq64 = np.asarray(bq, np.float64)

    M = wq64.T @ wk64                    # scores = x^T M x + (wk^T bq)^T x
    cvec = wk64.T @ bq64
    wvp = wp64 @ wv64                    # fused v+out projection
    U, S, Vt = np.linalg.svd(wvp)
    sA, sB = np.sqrt(S[0:128]), np.sqrt(S[128:255])
    VA = sA[:, None] * Vt[0:128]         # [128, C] pass-A v-side
    VB = sB[:, None] * Vt[128:255]       # [127, C] pass-B v-side
    UA = (U[:, 0:128] * sA).T            # [128, C] stationary (r, c)
    UB = np.zeros((P, C))
    # B-pass out partition 0 is the S ones-column; components sit on 1..127
    UB[1:128] = (U[:, 128:255] * sB).T
    wvab = np.vstack([VA, VB, cvec[None, :]])   # [256, C] fused projection
    bout = np.asarray(bp, np.float64) + wp64 @ np.asarray(bv, np.float64)

    shared = {
        "mk8": _pack_pairs(M.T.astype(np.float32)).astype(f8),
        "wvab8": _pack_pairs(wvab.T.astype(np.float32)).astype(f8),
        "uat16": np.ascontiguousarray(UA.astype(np.float32)).astype(bf),
        "ubt16": np.ascontiguousarray(UB.astype(np.float32)).astype(bf),
    }
    in_maps = []
    for core in range(NCORES):
        b, h = core // 2, core % 2
        xs = xf[b]
        if h == 1:  # roll so this core's query half is first (key order irrelevant)
            xs = np.concatenate([xs[:, NI:], xs[:, :NI]], axis=1)
        m = dict(shared)
        m["x8"] = _pack_pairs(xs).astype(f8)
        # residual with the output bias folded in: y = a*R + (x + bout)
        m["xq16"] = np.ascontiguousarray(
            xs[:, :NI] + bout[:, None].astype(np.float32)).astype(bf)
        in_maps.append(m)
    return in_maps


def _run(inputs, trace=False, **kwargs):
    from concourse.bass_utils import run_bass_kernel_spmd

    if "nc" not in _cache:
        _cache["nc"] = _build()
    nc = _cache["nc"]
    in_maps = _prep_inputs(**inputs)
    res = run_bass_kernel_spmd(
        nc, in_maps, core_ids=list(range(NCORES)), trace=trace, **kwargs
    )
    out = np.empty((B, C, N), np.float32)
    for core in range(NCORES):
        b, h = core // 2, core % 2
        out[b][:, h * NI:(h + 1) * NI] = res.results[core]["y"]
    return out.reshape(B, C, D, H, W), res


def kernel(**inputs):
    out, _ = _run(inputs)
    return out
